# revision 19
# baseline (speedup 1.0000x reference)
"""BitLinear (int2-packed weights, per-row int8 activation quant) on 8 trn2 cores.

Strategy (tensor-parallel over out_features, per sharding hint):
  - weight [16384, 1024] int8-packed -> 8 column shards of [2048, 1024]
  - x [4,2048,4096] f32 replicated to all cores as [8192, 4096]
  - per core: unpack int2 weights on device -> wT resident in SBUF (fp8e4,
    exact for {-2,-1,0,1}); per 128-token tile: absmax -> s -> quantize to
    integer-valued bf16 (magic-number round-half-even), DMA-xbar transpose
    q -> [K,M] layout, 32x4 bf16xfp8 matmuls accumulating exactly in fp32
    PSUM, dequant by ws0/s on ACT, store bf16.
  The weight prologue is interleaved with the first two token tiles, whose
  GEMMs run bank-outer so the PE starts once 4/16 weight tiles are ready.
"""

import os
import sys

if "/opt/trn_rl_repo" not in sys.path:
    sys.path.insert(0, "/opt/trn_rl_repo")

import numpy as np
import ml_dtypes

B, S, K, N = 4, 2048, 4096, 16384
NCORES = 8
NS = N // NCORES          # 2048 out_features per core
TT = 128                  # tokens per tile
NT = (B * S) // TT        # 64 token tiles
KT = K // 128             # 32 contraction tiles
NBANK = 512               # psum bank width (fp32)
NB = NS // NBANK          # 4 bank chunks
NTILE_W = (K // 4) // 128  # 8 packed-byte row tiles per core
MAGIC = 12582912.0        # 1.5 * 2**23: fp32 round-to-nearest-even trick

_CACHE = {}
LAST_RESULT = None


def _build_nc(debug_taps=False):
    from concourse import bacc, bass, tile, mybir

    dt = mybir.dt
    AF = mybir.ActivationFunctionType
    ALU = mybir.AluOpType

    nc = bacc.Bacc("TRN2", target_bir_lowering=False, debug=False,
                   num_devices=NCORES)

    x_d = nc.dram_tensor("x", [B * S, K], dt.float32, kind="ExternalInput")
    wp_d = nc.dram_tensor("wp", [K // 4, NS], dt.uint8, kind="ExternalInput")
    ws_d = nc.dram_tensor("ws", [4], dt.bfloat16, kind="ExternalInput")
    out_d = nc.dram_tensor("out", [B * S, NS], dt.bfloat16, kind="ExternalOutput")
    if debug_taps:
        s_dump = nc.dram_tensor("s_dump", [B * S, 1], dt.float32, kind="ExternalOutput")
        d_dump = nc.dram_tensor("d_dump", [B * S, 1], dt.float32, kind="ExternalOutput")
        q_dump = nc.dram_tensor("q_dump", [B * S, K], dt.bfloat16, kind="ExternalOutput")

    with tile.TileContext(nc) as tc:
        with (
            tc.tile_pool(name="wT", bufs=1) as wT_pool,
            tc.tile_pool(name="big", bufs=2) as big_pool,
            tc.tile_pool(name="qb", bufs=2) as qb_pool,
            tc.tile_pool(name="qT", bufs=3) as qT_pool,
            tc.tile_pool(name="outp", bufs=2) as out_pool,
            tc.tile_pool(name="wp", bufs=2) as wp_pool,
            tc.tile_pool(name="sc1", bufs=1) as sc1_pool,
            tc.tile_pool(name="sc2", bufs=3) as sc2_pool,
            tc.tile_pool(name="psum", bufs=2, space=bass.MemorySpace.PSUM) as ps_pool,
        ):
            # ---- constants / weight-scale broadcast ----
            magic = sc1_pool.tile([128, 1], dt.float32)
            nc.vector.memset(magic[:], MAGIC)

            ws_row = sc1_pool.tile([1, 4], dt.bfloat16)
            nc.gpsimd.dma_start(ws_row[:], ws_d[None, :])
            ws_f32 = sc1_pool.tile([1, 1], dt.float32)
            nc.vector.tensor_copy(ws_f32[:], ws_row[:, 0:1])
            ws_b = sc1_pool.tile([128, 1], dt.float32)
            nc.gpsimd.partition_broadcast(ws_b[:], ws_f32[:])

            # wT[p, kt, n]: value w[n, k] for k = 128*kt + p, resident fp8e4.
            wT = wT_pool.tile([128, KT, NS], dt.float8e4)

            negtwo = sc1_pool.tile([128, 1], dt.float32)
            nc.vector.memset(negtwo[:], -2.0)

            def emit_w_tile(a):
                """Unpack packed-byte rows kb in [128a, 128a+128) -> wT k-tiles
                kt = 8j + a, directly in [k-partition, n] layout (host
                pre-transposed the packed bytes, so no device transpose).
                Field j of byte kb is k = 4*kb + j."""
                wp_t = wp_pool.tile([128, NS], dt.uint8, tag="wp")
                nc.gpsimd.dma_start(wp_t[:], wp_d[128 * a : 128 * (a + 1), :])
                for j in range(4):
                    # per-byte field extract, 4 n-lanes per u32 ALU lane:
                    # fj = ((u32 >> 2j) & 0x03030303) ^ 0x02020202, then -2
                    tmp = wp_pool.tile([128, NS], dt.uint8, tag="tmpu")
                    if j == 0:
                        nc.vector.tensor_scalar(
                            tmp[:].bitcast(dt.uint32), wp_t[:].bitcast(dt.uint32),
                            0x03030303, 0x02020202,
                            op0=ALU.bitwise_and, op1=ALU.bitwise_xor,
                        )
                    else:
                        tmps = wp_pool.tile([128, NS], dt.uint8, tag="tmps")
                        nc.vector.tensor_scalar(
                            tmps[:].bitcast(dt.uint32), wp_t[:].bitcast(dt.uint32),
                            2 * j, None, op0=ALU.logical_shift_right,
                        )
                        nc.vector.tensor_scalar(
                            tmp[:].bitcast(dt.uint32), tmps[:].bitcast(dt.uint32),
                            0x03030303, 0x02020202,
                            op0=ALU.bitwise_and, op1=ALU.bitwise_xor,
                        )
                    # (fj - 2) -> fp8, split DVE / ACT to balance engines
                    if j < 3:
                        nc.vector.tensor_scalar(
                            wT[:, 8 * j + a, :], tmp[:], 2, None,
                            op0=ALU.subtract,
                        )
                    else:
                        nc.scalar.activation(
                            wT[:, 8 * j + a, :], tmp[:], AF.Identity,
                            bias=negtwo[:], scale=1.0,
                        )

            def emit_quant(t):
                """Load+quantize+transpose token tile t. Returns (qT, d)."""
                x_t = big_pool.tile([128, K], dt.float32, tag="x")
                eng = nc.sync if t < 2 else nc.scalar
                eng.dma_start(x_t[:], x_d[TT * t : TT * (t + 1), :])

                amax = sc2_pool.tile([128, 1], dt.float32, tag="amax")
                nc.vector.tensor_reduce(
                    amax[:], x_t[:], axis=mybir.AxisListType.X, op=ALU.max,
                    apply_absolute_value=True,
                )
                nc.vector.tensor_scalar_max(amax[:], amax[:], 1e-5)
                # s = 127/amax, d = ws0/s (reciprocal: TT divide has no ISA)
                r1 = sc2_pool.tile([128, 1], dt.float32, tag="r1")
                nc.vector.reciprocal(r1[:], amax[:])
                s_t = sc2_pool.tile([128, 1], dt.float32, tag="s")
                nc.vector.tensor_scalar_mul(s_t[:], r1[:], 127.0)
                rs = sc2_pool.tile([128, 1], dt.float32, tag="rs")
                nc.vector.reciprocal(rs[:], s_t[:])
                d_t = sc2_pool.tile([128, 1], dt.float32, tag="d")
                nc.vector.tensor_mul(d_t[:], ws_b[:], rs[:])

                # q = round_half_even(x * s), exact in bf16
                nc.scalar.activation(
                    x_t[:], x_t[:], AF.Identity, bias=magic[:], scale=s_t[:]
                )
                q_bf = qb_pool.tile([128, K], dt.bfloat16, tag="qb")
                nc.vector.tensor_scalar_sub(
                    q_bf[:].rearrange("t (j a p) -> t j a p", j=4, a=8, p=128),
                    x_t[:].rearrange("t (a p j) -> t j a p", a=8, p=128, j=4),
                    MAGIC,
                )

                # qT[p, kt, tt] = q[tt, 128*kt + p]
                qT = qT_pool.tile([128, KT, 128], dt.bfloat16, tag="qT")
                nc.sync.dma_start(qT[:], q_bf[:], transpose=True)

                if debug_taps:
                    nc.scalar.dma_start(s_dump[TT * t : TT * (t + 1), :], s_t[:])
                    nc.scalar.dma_start(d_dump[TT * t : TT * (t + 1), :], d_t[:])
                    nc.scalar.dma_start(q_dump[TT * t : TT * (t + 1), :], q_bf[:])
                return qT, d_t

            def emit_gemm_bank(acc, qT, nb):
                for kt in range(KT):
                    nc.tensor.matmul(
                        acc[:, NBANK * nb : NBANK * (nb + 1)],
                        qT[:, kt, :],
                        wT[:, kt, NBANK * nb : NBANK * (nb + 1)],
                        start=(kt == 0),
                        stop=(kt == KT - 1),
                    )

            def emit_gemm(acc, qT):
                for kt in range(KT):
                    for nb in range(NB):
                        nc.tensor.matmul(
                            acc[:, NBANK * nb : NBANK * (nb + 1)],
                            qT[:, kt, :],
                            wT[:, kt, NBANK * nb : NBANK * (nb + 1)],
                            start=(kt == 0),
                            stop=(kt == KT - 1),
                        )

            def emit_store(t, acc, d_t):
                out_t = out_pool.tile([128, NS], dt.bfloat16)
                nc.scalar.mul(out_t[:], acc[:], d_t[:])
                nc.scalar.dma_start(out_d[TT * t : TT * (t + 1), :], out_t[:])

            # ---- warmup: first two token tiles, GEMM trickled per a-tile.
            # DVE order matters: q0 chain, first weight tile, q1 chain, rest.
            emit_w_tile(0)
            qT0, d0 = emit_quant(0)
            qT1, d1 = emit_quant(1)
            acc0 = ps_pool.tile([128, NS], dt.float32, tag="acc")
            acc1 = ps_pool.tile([128, NS], dt.float32, tag="acc")

            def warm_mm(acc, qT, a):
                for j in range(4):
                    kt = 8 * j + a
                    for nb in range(NB):
                        nc.tensor.matmul(
                            acc[:, NBANK * nb : NBANK * (nb + 1)],
                            qT[:, kt, :],
                            wT[:, kt, NBANK * nb : NBANK * (nb + 1)],
                            start=(a == 0 and j == 0),
                            stop=(a == NTILE_W - 1 and j == 3),
                        )

            warm_mm(acc0, qT0, 0)
            warm_mm(acc1, qT1, 0)
            for a in range(1, NTILE_W):
                emit_w_tile(a)
                warm_mm(acc0, qT0, a)
                warm_mm(acc1, qT1, a)
            emit_store(0, acc0, d0)
            emit_store(1, acc1, d1)

            # ---- steady state ----
            for t in range(2, NT):
                qT, d_t = emit_quant(t)
                acc = ps_pool.tile([128, NS], dt.float32, tag="acc")
                emit_gemm(acc, qT)
                emit_store(t, acc, d_t)

    nc.compile()
    return nc


def _get_nc():
    if "nc" not in _CACHE:
        _CACHE["nc"] = _build_nc()
    return _CACHE["nc"]


def _install_profile_shims():
    """Optional NTFF profiling support (the container's antenv lacks
    axon_hooks). Only used when BITLIN_TRACE=1."""
    import types
    import ctypes
    import contextlib

    if "antenv.axon_hooks" in sys.modules:
        return
    so_path = "/opt/axon/libaxon_pjrt.so"
    lib = ctypes.CDLL(so_path)
    lib.axon_start_nrt_profile.argtypes = [
        ctypes.POINTER(ctypes.c_int64), ctypes.c_size_t,
    ]
    lib.axon_start_nrt_profile.restype = ctypes.c_int64
    lib.axon_stop_nrt_profile.argtypes = [ctypes.c_char_p]
    lib.axon_stop_nrt_profile.restype = ctypes.c_int64

    @contextlib.contextmanager
    def _hook(output_dir, device_ids):
        import jax

        jax.devices()
        if device_ids:
            ids = (ctypes.c_int64 * len(device_ids))(*device_ids)
            rc = lib.axon_start_nrt_profile(ids, len(device_ids))
        else:
            rc = lib.axon_start_nrt_profile(None, 0)
        if rc != 0:
            raise RuntimeError(f"axon_start_nrt_profile rc={rc}")
        try:
            yield
        finally:
            n = lib.axon_stop_nrt_profile(str(output_dir).encode())
            print(f"ntff profile: {n} file(s) in {output_dir}", file=sys.stderr)

    mod = types.ModuleType("antenv.axon_hooks")
    mod.get_axon_ntff_profile_hook = lambda: _hook
    mod.set_axon_ntff_profile_hook = lambda h: None
    import antenv

    sys.modules["antenv.axon_hooks"] = mod
    antenv.axon_hooks = mod

    from concourse import bass_utils

    bass_utils.upload_artifacts = lambda tmpdir: "(upload disabled)"


def kernel(x, weight, weight_scale):
    global LAST_RESULT
    from concourse.bass_utils import run_bass_kernel_spmd

    x = np.asarray(x, dtype=np.float32).reshape(B * S, K)
    weight = np.asarray(weight, dtype=np.int8)
    ws = np.asarray(weight_scale, dtype=ml_dtypes.bfloat16)

    trace = os.environ.get("BITLIN_TRACE", "") == "1"
    if trace:
        _install_profile_shims()

    nc = _get_nc()
    in_maps = []
    for c in range(NCORES):
        wp = np.ascontiguousarray(weight[NS * c : NS * (c + 1), :].T).view(np.uint8)
        in_maps.append({"x": x, "wp": wp, "ws": ws})

    res = run_bass_kernel_spmd(
        nc, in_maps, core_ids=list(range(NCORES)), trace=trace
    )
    LAST_RESULT = res
    out = np.concatenate(
        [res.results[c]["out"] for c in range(NCORES)], axis=1
    )
    return out.reshape(B, S, N)
